# revision 2
# baseline (speedup 1.0000x reference)
"""v6: fp16, transposed layout, fused per-row multiply, aligned 2x/4x APs.

Per core: partitions = 128 output cols (j). Host lays the kernel tap axis
out as k'' = v*20 + u (19 v-rows of 20 slots, u=19 slot zero; rows 380..383
zero) so every innermost run is 20 elements (even) and every base offset is
4-byte aligned -- the conditions for DVE 2x (tensor_tensor) and 4x
(tensor_scalar) fp16 perf modes on real silicon.

Kernel tiles [k'', j] stream HBM->SBUF through the DMA xbar transpose in
batched form: one dma_start_transpose per (128-tap chunk, 16-row block)
yields kerT[j, ii, k''].

x: transposed + fp16 on host; device holds TWO sliding col-windows of it,
the second shifted one row, so the fused multiply's innermost run start
(i or i-1) is always even.

Per output row i: ONE DVE tensor_tensor (2x fp16) computes all 3*380
products; per (c, i): either a DVE tensor_scalar (4x fp16, scale=1/361,
accum_out) or a ScalarE activation (Copy, scale, accum_out) reduces 380
products into the fp32 accumulator. Output transposed back via PE.
"""

import numpy as np

import concourse.bacc as bacc
import concourse.mybir as mybir
import concourse.tile as tile
from concourse import bass_utils
from concourse.ap import AP

L = 19
K2 = L * L
VS = 20            # padded v-row stride (taps per v-row incl. zero slot)
NT = L * VS        # 380 product slots per channel
K2P = 384          # padded tap rows in HBM (3 x 128)
PAD = L // 2
B, C, H, W = 2, 3, 256, 256
BLK = 128
XS = BLK + L - 1   # 146 valid cols
XSP = 148          # padded row stride (even)
IB = 16            # i-rows per batched transpose DMA

_CACHE = {}
LAST_EXEC_NS = None


def _emit(nc, xT_d, k_d, ident_d, o_d, tc):
    f16 = mybir.dt.float16
    f32 = mybir.dt.float32
    with (
        tc.tile_pool(name="xwp", bufs=1) as xwp,
        tc.tile_pool(name="idp", bufs=1) as idp,
        tc.tile_pool(name="kerTp", bufs=3) as kerTp,
        tc.tile_pool(name="prp", bufs=8) as prp,
        tc.tile_pool(name="scp", bufs=8) as scp,
        tc.tile_pool(name="obp", bufs=1) as obp,
        tc.tile_pool(name="otp", bufs=3) as otp,
        tc.tile_pool(name="psp", bufs=3, space="PSUM") as psp,
    ):
        # Block list: two small leading blocks shrink the startup ramp.
        blocks = [(0, 4), (4, 16)] + [(b, b + IB) for b in range(16, BLK, IB)]

        def emit_kerT(b0, b1):
            t = kerTp.tile([BLK, IB * K2P], f16, tag="kerT")
            t4 = t.rearrange("p (e k) -> p e k", e=b1 - b0)
            for ch in range(K2P // BLK):
                nc.sync.dma_start_transpose(
                    out=t4[:, :, ch * BLK:(ch + 1) * BLK],
                    in_=k_d[ch * BLK:(ch + 1) * BLK, b0:b1, :])
            return t4

        preT = {}
        preT[blocks[0]] = emit_kerT(*blocks[0])

        # Sliding col-windows of transposed x:
        # xwE[p, c, v, r] = xpad[c, r,   p+v]   (even-i reads start at r=i)
        # xwO[p, c, v, r] = xpad[c, r+1, p+v]   (odd-i reads start at r=i-1)
        xwinE = xwp.tile([BLK, C * L * XSP], f16, tag="xwE")
        xwinO = xwp.tile([BLK, C * L * XSP], f16, tag="xwO")
        xwE = xwinE.rearrange("p (c v r) -> p c v r", c=C, v=L, r=XSP)
        xwO = xwinO.rearrange("p (c v r) -> p c v r", c=C, v=L, r=XSP)
        # xT_d is [C, XSP(col), XSP(row)] host-padded with zeros; build each
        # window copy with 3 big r-chunked DMAs so compute can start early.
        # dest[p, c, v, r] = xT[c, p+v, r (+1 for the odd copy)] -- the v dim
        # overlaps the partition dim (same stride), built as a manual AP.
        echunks = ((0, 48), (48, 96), (96, XSP))
        ochunks = ((0, 47), (47, 95), (95, XS + 1))
        for n, (r0, r1) in enumerate(echunks):
            for c in range(C):
                src = AP(xT_d.tensor, c * XSP * XSP + r0,
                         [[XSP, BLK], [XSP, L], [1, r1 - r0]])
                nc.sync.dma_start(out=xwE[:, c, :, r0:r1], in_=src)
            # Shifted copy for odd rows on the otherwise-idle GpSimd engine
            # (strides are free for engines; runs during the DMA ramp).
            o0, o1 = ochunks[n]
            nc.gpsimd.tensor_copy(xwO[:, :, :, o0:o1],
                                  xwE[:, :, :, o0 + 1:o1 + 1])
            if n == 0:
                preT[blocks[1]] = emit_kerT(*blocks[1])
            elif n == 1:
                preT[blocks[2]] = emit_kerT(*blocks[2])

        ident = idp.tile([BLK, BLK], f32)
        nc.sync.dma_start(out=ident[:, :], in_=ident_d)

        out_sb = obp.tile([BLK, C * BLK], f32)
        ob3 = out_sb.rearrange("p (c i) -> p c i", c=C)

        for (b0, b1) in blocks:
            kerT4 = preT.get((b0, b1)) or emit_kerT(b0, b1)
            for ii in range(b1 - b0):
                i = b0 + ii
                if i % 2 == 0:
                    xsl = xwE[:, :, :, i:i + VS]
                else:
                    xsl = xwO[:, :, :, i - 1:i - 1 + VS]
                k3 = kerT4[:, ii, 0:NT].rearrange("p (v u) -> p v u", v=L)
                kb = k3.unsqueeze(1).broadcast_to([BLK, C, L, VS])
                prod = prp.tile([BLK, C * NT], f16, tag="prod")
                pr4 = prod.rearrange("p (c v u) -> p c v u", c=C, v=L)
                # ONE fused multiply for all channels: 2x fp16 TT.
                nc.vector.tensor_tensor(
                    out=pr4, in0=xsl, in1=kb, op=mybir.AluOpType.mult)
                pr2 = prod.rearrange("p (c t) -> p c t", c=C)
                for c in range(C):
                    # ~1.67 of 3 reduces on DVE (tensor_scalar 4x), rest ACT.
                    on_dve = (c == 0) or (c == 1 and i % 3 != 0)
                    if on_dve:
                        scr = scp.tile([BLK, NT], f16, tag="scr")
                        nc.vector.tensor_scalar(
                            out=scr[:, :],
                            in0=pr2[:, c, :],
                            scalar1=1.0 / K2,
                            scalar2=None,
                            op0=mybir.AluOpType.mult,
                            op1=mybir.AluOpType.add,
                            accum_out=ob3[:, c, i:i + 1],
                        )
                    else:
                        scr = scp.tile([BLK, NT], f16, tag="scr")
                        nc.scalar.activation(
                            out=scr[:, :],
                            in_=pr2[:, c, :],
                            func=mybir.ActivationFunctionType.Copy,
                            scale=1.0 / K2,
                            accum_out=ob3[:, c, i:i + 1],
                        )

        # Transpose [j, (c, i)] -> [i, (c, j)] via PE, then clean DMAs.
        for c in range(C):
            ps = psp.tile([BLK, BLK], f32, tag="ps")
            nc.tensor.transpose(ps[:, :], ob3[:, c, :], ident[:, :])
            ot = otp.tile([BLK, BLK], f32, tag="ot")
            nc.scalar.copy(out=ot[:, :], in_=ps[:, :])
            nc.sync.dma_start(out=o_d[c], in_=ot[:, :])


def build_program():
    if "nc" in _CACHE:
        return _CACHE["nc"]
    nc = bacc.Bacc(
        "TRN2",
        target_bir_lowering=False,
        debug=False,
        enable_asserts=True,
        num_devices=8,
    )
    f16 = mybir.dt.float16
    f32 = mybir.dt.float32
    xT_d = nc.dram_tensor("xT", [C, XSP, XSP], f16,
                          kind="ExternalInput").ap()
    k_d = nc.dram_tensor("ker", [K2P, BLK, BLK], f16, kind="ExternalInput").ap()
    ident_d = nc.dram_tensor("ident", [BLK, BLK], f32,
                             kind="ExternalInput").ap()
    o_d = nc.dram_tensor("out", [C, BLK, BLK], f32, kind="ExternalOutput").ap()
    with tile.TileContext(nc) as tc:
        _emit(nc, xT_d, k_d, ident_d, o_d, tc)
    nc.compile()
    _CACHE["nc"] = nc
    return nc


def shard_inputs(input, kernel):
    xpad = np.pad(input, ((0, 0), (0, 0), (PAD, PAD), (PAD, PAD)),
                  mode="reflect")
    ident = np.eye(BLK, dtype=np.float32)
    # dest row k'' = v*20 + u  <-  source row u*19 + v (u < 19), else zero
    in_maps = []
    for core in range(8):
        b, hh, wh = core >> 2, (core >> 1) & 1, core & 1
        xs = xpad[b, :, hh * BLK:hh * BLK + XS, wh * BLK:wh * BLK + XS]
        xT = np.zeros((C, XSP, XSP), dtype=np.float16)
        xT[:, :XS, :XS] = xs.transpose(0, 2, 1).astype(np.float16)
        ks = kernel[b, :, hh * BLK:(hh + 1) * BLK, wh * BLK:(wh + 1) * BLK]
        ksp = np.zeros((K2P, BLK, BLK), dtype=np.float16)
        src = ks.astype(np.float16).reshape(L, L, BLK, BLK)  # [u, v, i, j]
        for v in range(L):
            ksp[v * VS:v * VS + L] = src[:, v]
        in_maps.append({"xT": xT, "ker": ksp, "ident": ident})
    return in_maps


def gather_outputs(results):
    out = np.empty((B, C, H, W), dtype=np.float32)
    for core in range(8):
        b, hh, wh = core >> 2, (core >> 1) & 1, core & 1
        out[b, :, hh * BLK:(hh + 1) * BLK, wh * BLK:(wh + 1) * BLK] = \
            results[core]["out"]
    return out


def kernel(input, kernel):
    global LAST_EXEC_NS
    nc = build_program()
    in_maps = shard_inputs(np.asarray(input, dtype=np.float32),
                           np.asarray(kernel, dtype=np.float32))
    res = bass_utils.run_bass_kernel_spmd(
        nc, in_maps, core_ids=list(range(8)))
    LAST_EXEC_NS = res.exec_time_ns
    return gather_outputs(res.results)
